# Initial kernel scaffold
#
"""Expert-parallel MoE kernel for Trainium2 (8 NeuronCores, 1 expert per core).

Strategy:
  - Host computes routing (top-k affinity normalization + combine weights) and
    gathers each expert's tokens; core e processes expert e's routed tokens only
    (~T*K/E = 1024 tokens instead of dense T=4096).
  - Quantized weights are uploaded as CENTERED integer codes (q-128) in fp16
    (exactly representable), per-output-channel scales are applied on-chip
    AFTER the matmul, so no dequantization error on weights.
  - Matmuls keep weights as the stationary operand; activations/intermediates
    flow as [channel_partition, token_free] tiles so gate_up -> glu -> down
    chains with zero transposes.
  - Combine weights are folded into the GLU epilogue; host scatter-adds the
    per-expert outputs back to the full [T, H] output.
"""

import math
from contextlib import ExitStack

import numpy as np

import concourse.bass as bass
import concourse.tile as tile
import concourse.mybir as mybir
from concourse import bacc
from concourse.bass_utils import run_bass_kernel_spmd

E, H, I, TOPK = 8, 4096, 1792, 2
ZP = 128.0
P = 128
KH = H // P          # 32 contraction slabs for gate_up
KI = I // P          # 14 contraction slabs for down
NJ = I // P          # 14 gate/up pair groups (each 128 gate + 128 up cols)
NG = (H // P) // 2   # 16 down output groups (each 256 out cols)

fp16 = mybir.dt.float16
fp32 = mybir.dt.float32


def build_moe_nc(C, num_devices=8, h=H, i_dim=I, W=None):
    """Build + compile the per-core MoE bass program for token capacity C.
    C = tc_chunks * W; W (chunk width, <=512 fp32 PSUM columns) defaults to 512."""
    kh, ki = h // P, i_dim // P
    nj, ng = i_dim // P, (h // P) // 2
    if W is None:
        W = min(C, 512)
    tc_chunks = C // W
    assert C % W == 0 and W <= 512

    nc = bacc.Bacc("TRN2", target_bir_lowering=False, debug=False,
                   num_devices=num_devices)
    xT = nc.dram_tensor("xT", [tc_chunks, P, kh, W], fp16, kind="ExternalInput").ap()
    wgu = nc.dram_tensor("wgu", [nj, P, kh, 256], fp16, kind="ExternalInput").ap()
    wd = nc.dram_tensor("wd", [ng, P, ki, 256], fp16, kind="ExternalInput").ap()
    sgu = nc.dram_tensor("sgu", [P, 2 * nj], fp32, kind="ExternalInput").ap()
    sd = nc.dram_tensor("sd", [P, 2 * ng], fp32, kind="ExternalInput").ap()
    wcomb = nc.dram_tensor("wcomb", [P, C], fp32, kind="ExternalInput").ap()
    out = nc.dram_tensor("out", [P, h // P, C], fp32, kind="ExternalOutput").ap()

    with tile.TileContext(nc) as tcx, ExitStack() as ctx:
        const_pool = ctx.enter_context(tcx.tile_pool(name="const", bufs=1))
        wpool = ctx.enter_context(tcx.tile_pool(name="w", bufs=3))
        hpool = ctx.enter_context(tcx.tile_pool(name="h", bufs=1))
        tmp_pool = ctx.enter_context(tcx.tile_pool(name="tmp", bufs=3))
        out_pool = ctx.enter_context(tcx.tile_pool(name="outp", bufs=3))
        psum_pool = ctx.enter_context(tcx.tile_pool(name="psum", bufs=8, space="PSUM"))

        # Fill-phase DMA plan. PE needs ~10.5MB (xT + wgu0) before the first
        # group can finish; per-dma_start BW is ~138GB/s at 64KB but ~341+ at
        # >=1MB, so: a few tiny slab DMAs to let the PE start at ~9us, then
        # big contiguous DMAs for the bulk, with wgu[1] issued before xT
        # chunk 1 so group 1's weights are in flight during group 0.
        # xT_sb is chunk-major [P, tc, kh, 512] so every xT DMA lands in
        # contiguous SBUF (big descriptor elements, full DMA rate).
        xT_sb = const_pool.tile([P, tc_chunks, kh, W], fp16)
        wt0 = wpool.tile([P, kh, 256], fp16, tag="wgu")
        wt1 = wpool.tile([P, kh, 256], fp16, tag="wgu")

        # Warm up the PE clock (HAM un-throttle needs ~3.4us of PE-busy)
        # during the DMA fill bubble with dependency-free dummy matmuls.
        dummy_w = const_pool.tile([P, P], fp16)
        nc.vector.memset(dummy_w[:], 1.0)
        dummy_x = const_pool.tile([P, W], fp16)
        nc.vector.memset(dummy_x[:], 1.0)
        dummy_ps = psum_pool.tile([P, W], fp32, tag="ps", name="dummy_ps")
        # 6 dummies (~2.5us cold) bridge until the first real slab lands;
        # the real matmul stream then keeps the HAM activity window busy.
        for _ in range(6):
            nc.tensor.matmul(dummy_ps[:], dummy_w[:], dummy_x[:],
                             start=True, stop=True)

        # Slab-laddered fill: fine blocks first so matmuls start early,
        # coarser later to respect the ~0.6us/DMA sequencer issue cost.
        ladder, _a, _w = [], 0, 1
        while _a < kh:
            _b = min(kh, _a + _w)
            ladder.append((_a, _b))
            _a, _w = _b, _w * 2
        if len(ladder) > 1:
            ladder[-1] = (ladder[-1][0], kh)
        sgu_sb = const_pool.tile([P, 2 * nj], fp32)
        sd_sb = const_pool.tile([P, 2 * ng], fp32)
        wc_sb = const_pool.tile([P, C], fp32)
        # Dual HWDGE issue, balanced: sync ring carries wgu0 + even xT
        # chunks, scalar ring carries odd xT chunks + wgu1 — halves the
        # serial ~0.6us/DMA issue latency chain on each ring.
        for i, (a, b) in enumerate(ladder):
            nc.sync.dma_start(wt0[:, a:b], wgu[0, :, a:b])
            for t in range(tc_chunks):
                eng = nc.scalar if t % 2 == 0 else nc.sync
                eng.dma_start(xT_sb[:, t, a:b], xT[t, :, a:b])
            if i == 1 or (i == 0 and len(ladder) == 1):
                nc.scalar.dma_start(sgu_sb[:], sgu[:])
            if i >= 2:
                # trail group 1's weight ladder two blocks behind the fill so
                # its early slabs land before group 1's matmuls need them
                a1, b1 = ladder[i - 2]
                nc.scalar.dma_start(wt1[:, a1:b1], wgu[1, :, a1:b1])
        for a1, b1 in ladder[max(0, len(ladder) - 2):]:
            nc.scalar.dma_start(wt1[:, a1:b1], wgu[1, :, a1:b1])
        # wcomb is epilogue-only (not PE-blocking until group 0's psums must
        # recycle at ~group 2); keep its 0.5MB out of the PE-critical fill
        nc.scalar.dma_start(wc_sb[:], wcomb[:])
        nc.sync.dma_start(sd_sb[:], sd[:])

        h_sb = hpool.tile([P, ki, C], fp16)

        # ---- gate_up matmul + SiLU GLU (combine weight folded in) ----
        for j in range(nj):
            if j == 0:
                wt = wt0
            elif j == 1:
                wt = wt1
            else:
                wt = wpool.tile([P, kh, 256], fp16, tag="wgu")
                nc.sync.dma_start(wt[:], wgu[j])
            # k-major across the batch's (t-chunk, gate/up) psums: each
            # matmul only needs k-slab k of its operands, matching slab
            # arrival order. Batches of <=3 t-chunks keep psum tile demand
            # <=6 of the pool's 8 slots (deadlock-free for any tc_chunks).
            for t0_b in range(0, tc_chunks, 3):
              tb = list(range(t0_b, min(t0_b + 3, tc_chunks)))
              pss = {t: (psum_pool.tile([P, W], fp32, tag="ps", name=f"psg{t}"),
                         psum_pool.tile([P, W], fp32, tag="ps", name=f"psu{t}"))
                     for t in tb}
              for k in range(kh):
                for t in tb:
                    nc.tensor.matmul(pss[t][0][:], wt[:, k, 0:P],
                                     xT_sb[:, t, k],
                                     start=(k == 0), stop=(k == kh - 1))
                    nc.tensor.matmul(pss[t][1][:], wt[:, k, P:2 * P],
                                     xT_sb[:, t, k],
                                     start=(k == 0), stop=(k == kh - 1))
              for t in tb:
                ts = slice(t * W, (t + 1) * W)
                ps_g, ps_u = pss[t]
                # h = sigmoid(g*sg) * g * u * (sg*su) * wcomb
                # (col 2j of sgu holds sg; col 2j+1 holds sg*su)
                act = tmp_pool.tile([P, W], fp32, tag="act")
                nc.scalar.activation(act[:], ps_g[:],
                                     mybir.ActivationFunctionType.Sigmoid,
                                     scale=sgu_sb[:, 2 * j:2 * j + 1])
                m1 = tmp_pool.tile([P, W], fp32, tag="m1")
                nc.vector.tensor_mul(m1[:], act[:], ps_u[:])
                nc.vector.tensor_mul(m1[:], m1[:], ps_g[:])
                nc.vector.tensor_scalar_mul(m1[:], m1[:],
                                            sgu_sb[:, 2 * j + 1:2 * j + 2])
                nc.vector.tensor_tensor(h_sb[:, j, ts], m1[:], wc_sb[:, ts],
                                        mybir.AluOpType.mult)

        # ---- down matmul + per-channel scale ----
        for g in range(ng):
            wdt = wpool.tile([P, ki, 256], fp16, tag="wd")
            nc.sync.dma_start(wdt[:], wd[g])
            for half in range(2):
                m = 2 * g + half
                ot = out_pool.tile([P, C], fp32, tag="ot")
                for t in range(tc_chunks):
                    ts = slice(t * W, (t + 1) * W)
                    ps = psum_pool.tile([P, W], fp32, tag="ps")
                    for k in range(ki):
                        nc.tensor.matmul(ps[:], wdt[:, k, half * P:(half + 1) * P],
                                         h_sb[:, k, ts],
                                         start=(k == 0), stop=(k == ki - 1))
                    nc.vector.tensor_scalar_mul(ot[:, ts], ps[:], sd_sb[:, m:m + 1])
                    nc.scalar.dma_start(out[:, m, ts], ot[:, ts])

    nc.compile()
    return nc


_NC_CACHE = {}


def _get_nc(C, W):
    key = (C, W)
    if key not in _NC_CACHE:
        _NC_CACHE[key] = build_moe_nc(C, W=W)
    return _NC_CACHE[key]


def _prep_core_inputs(e, C, W, hidden, combine, gate_up_w_q, gate_up_scale,
                      down_w_q, down_scale):
    """Build the device input map for expert e. Returns (in_map, token_ids)."""
    ids = np.nonzero(combine[:, e])[0]
    n = len(ids)
    tc_chunks = C // W

    xTf = np.zeros((H, C), np.float16)
    if n:
        xTf[:, :n] = hidden[ids].T.astype(np.float16)
    xT_dev = np.ascontiguousarray(
        xTf.reshape(KH, P, tc_chunks, W).transpose(2, 1, 0, 3))

    wgu_c = (gate_up_w_q[e].astype(np.int16) - 128).astype(np.float16)  # [H, 2I]
    wg = wgu_c[:, :I].reshape(H, NJ, P)
    wu = wgu_c[:, I:].reshape(H, NJ, P)
    pairs = np.concatenate([wg, wu], axis=2)                       # [H, NJ, 256]
    wgu_dev = np.ascontiguousarray(
        pairs.reshape(KH, P, NJ, 256).transpose(2, 1, 0, 3))       # [NJ,128,KH,256]

    wd_c = (down_w_q[e].astype(np.int16) - 128).astype(np.float16)  # [I, H]
    wd_dev = np.ascontiguousarray(
        wd_c.reshape(KI, P, NG, 256).transpose(2, 1, 0, 3))        # [NG,128,KI,256]

    sg = gate_up_scale[e, 0, :I].reshape(NJ, P).astype(np.float32)
    su = gate_up_scale[e, 0, I:].reshape(NJ, P).astype(np.float32)
    sgu_dev = np.empty((P, 2 * NJ), np.float32)
    sgu_dev[:, 0::2] = sg.T
    sgu_dev[:, 1::2] = (sg * su).T

    sd_dev = np.ascontiguousarray(
        down_scale[e, 0].reshape(H // P, P).T.astype(np.float32))  # [128, 32]

    wvec = np.zeros(C, np.float32)
    if n:
        wvec[:n] = combine[ids, e]
    wcomb_dev = np.ascontiguousarray(np.broadcast_to(wvec[None, :], (P, C)))

    return dict(xT=xT_dev, wgu=wgu_dev, wd=wd_dev, sgu=sgu_dev, sd=sd_dev,
                wcomb=wcomb_dev), ids


def host_routing(expert_affinities, expert_index):
    """Top-k affinity normalization -> dense combine matrix [T, E]."""
    T = expert_index.shape[0]
    sel = np.take_along_axis(expert_affinities.astype(np.float32),
                             expert_index, axis=1)
    sel = sel / sel.sum(axis=1, keepdims=True)
    combine = np.zeros((T, E), np.float32)
    np.add.at(combine,
              (np.repeat(np.arange(T), expert_index.shape[1]),
               expert_index.ravel()),
              sel.ravel())
    return combine


def kernel(hidden_states, expert_affinities, gate_up_w_q, gate_up_scale,
           down_w_q, down_scale, expert_index, seq_len=None, **_unused):
    hidden = np.asarray(hidden_states, dtype=np.float32)
    aff = np.asarray(expert_affinities, dtype=np.float32)
    ei = np.asarray(expert_index, dtype=np.int64)
    gq = np.asarray(gate_up_w_q)
    gs = np.asarray(gate_up_scale, dtype=np.float32)
    dq = np.asarray(down_w_q)
    ds = np.asarray(down_scale, dtype=np.float32)
    T = hidden.shape[0]

    combine = host_routing(aff, ei)
    counts = (combine > 0).sum(axis=0)
    cmax = max(2, int(counts.max()))
    tc = max(1, int(math.ceil(cmax / 512)))
    Wc = int(math.ceil(cmax / (2 * tc))) * 2   # even chunk width
    C = tc * Wc

    nc = _get_nc(C, Wc)

    in_maps = []
    all_ids = []
    for e in range(E):
        im, ids = _prep_core_inputs(e, C, Wc, hidden, combine, gq, gs, dq, ds)
        in_maps.append(im)
        all_ids.append(ids)

    res = run_bass_kernel_spmd(nc, in_maps, list(range(E)))

    y = np.zeros((T, H), np.float32)
    for e in range(E):
        ids = all_ids[e]
        if len(ids) == 0:
            continue
        out_dev = res.results[e]["out"]            # [128, 32, C]
        out_full = out_dev.transpose(1, 0, 2).reshape(H, C)
        y[ids] += out_full[:, :len(ids)].T
    return y



# revision 1
# speedup vs baseline: 1.1868x; 1.1868x over previous
"""Expert-parallel MoE kernel for Trainium2 (8 NeuronCores, 1 expert per core).

Strategy:
  - Host computes routing (top-k affinity normalization + combine weights) and
    gathers each expert's tokens; core e processes expert e's routed tokens only
    (~T*K/E = 1024 tokens instead of dense T=4096).
  - Quantized weights are uploaded as CENTERED integer codes (q-128) in fp16
    (exactly representable), per-output-channel scales are applied on-chip
    AFTER the matmul, so no dequantization error on weights.
  - Matmuls keep weights as the stationary operand; activations/intermediates
    flow as [channel_partition, token_free] tiles so gate_up -> glu -> down
    chains with zero transposes.
  - Combine weights are folded into the GLU epilogue; host scatter-adds the
    per-expert outputs back to the full [T, H] output.
"""

import math
from contextlib import ExitStack

import numpy as np

import concourse.bass as bass
import concourse.tile as tile
import concourse.mybir as mybir
from concourse import bacc
from concourse.bass_utils import run_bass_kernel_spmd

E, H, I, TOPK = 8, 4096, 1792, 2
ZP = 128.0
P = 128
KH = H // P          # 32 contraction slabs for gate_up
KI = I // P          # 14 contraction slabs for down
NJ = I // P          # 14 gate/up pair groups (each 128 gate + 128 up cols)
NG = (H // P) // 2   # 16 down output groups (each 256 out cols)

fp16 = mybir.dt.float16
fp32 = mybir.dt.float32


def build_moe_nc(C, num_devices=8, h=H, i_dim=I, W=None):
    """Build + compile the per-core MoE bass program for token capacity C.
    C = tc_chunks * W; W (chunk width, <=512 fp32 PSUM columns) defaults to 512."""
    kh, ki = h // P, i_dim // P
    nj, ng = i_dim // P, (h // P) // 2
    if W is None:
        W = min(C, 512)
    tc_chunks = C // W
    assert C % W == 0 and W <= 512

    nc = bacc.Bacc("TRN2", target_bir_lowering=False, debug=False,
                   num_devices=num_devices)
    xT = nc.dram_tensor("xT", [tc_chunks, P, kh, W], fp16, kind="ExternalInput").ap()
    wgu = nc.dram_tensor("wgu", [nj, P, kh, 256], fp16, kind="ExternalInput").ap()
    wd = nc.dram_tensor("wd", [ng, P, ki, 256], fp16, kind="ExternalInput").ap()
    sgu = nc.dram_tensor("sgu", [P, 2 * nj], fp32, kind="ExternalInput").ap()
    sd = nc.dram_tensor("sd", [P, 2 * ng], fp32, kind="ExternalInput").ap()
    wcomb = nc.dram_tensor("wcomb", [P, C], fp32, kind="ExternalInput").ap()
    out = nc.dram_tensor("out", [P, h // P, C], fp32, kind="ExternalOutput").ap()

    with tile.TileContext(nc) as tcx, ExitStack() as ctx:
        const_pool = ctx.enter_context(tcx.tile_pool(name="const", bufs=1))
        wpool = ctx.enter_context(tcx.tile_pool(name="w", bufs=3))
        hpool = ctx.enter_context(tcx.tile_pool(name="h", bufs=1))
        tmp_pool = ctx.enter_context(tcx.tile_pool(name="tmp", bufs=3))
        out_pool = ctx.enter_context(tcx.tile_pool(name="outp", bufs=3))
        psum_pool = ctx.enter_context(tcx.tile_pool(name="psum", bufs=8, space="PSUM"))

        # Fill-phase DMA plan. PE needs ~10.5MB (xT + wgu0) before the first
        # group can finish; per-dma_start BW is ~138GB/s at 64KB but ~341+ at
        # >=1MB, so: a few tiny slab DMAs to let the PE start at ~9us, then
        # big contiguous DMAs for the bulk, with wgu[1] issued before xT
        # chunk 1 so group 1's weights are in flight during group 0.
        # xT_sb is chunk-major [P, tc, kh, 512] so every xT DMA lands in
        # contiguous SBUF (big descriptor elements, full DMA rate).
        xT_sb = const_pool.tile([P, tc_chunks, kh, W], fp16)
        wt0 = wpool.tile([P, kh, 256], fp16, tag="wgu")
        wt1 = wpool.tile([P, kh, 256], fp16, tag="wgu")

        # Warm up the PE clock (HAM un-throttle needs ~3.4us of PE-busy)
        # during the DMA fill bubble with dependency-free dummy matmuls.
        dummy_w = const_pool.tile([P, P], fp16)
        nc.vector.memset(dummy_w[:], 1.0)
        dummy_x = const_pool.tile([P, W], fp16)
        nc.vector.memset(dummy_x[:], 1.0)
        dummy_ps = psum_pool.tile([P, W], fp32, tag="ps", name="dummy_ps")
        # 6 dummies (~2.5us cold) bridge until the first real slab lands;
        # the real matmul stream then keeps the HAM activity window busy.
        for _ in range(6):
            nc.tensor.matmul(dummy_ps[:], dummy_w[:], dummy_x[:],
                             start=True, stop=True)

        # Slab-laddered fill: fine blocks first so matmuls start early,
        # coarser later to respect the ~0.6us/DMA sequencer issue cost.
        ladder, _a, _w = [], 0, 1
        while _a < kh:
            _b = min(kh, _a + _w)
            ladder.append((_a, _b))
            _a, _w = _b, _w * 2
        if len(ladder) > 1:
            ladder[-1] = (ladder[-1][0], kh)
        sgu_sb = const_pool.tile([P, 2 * nj], fp32)
        sd_sb = const_pool.tile([P, 2 * ng], fp32)
        wc_sb = const_pool.tile([P, C], fp32)
        # Dual HWDGE issue, balanced: sync ring carries wgu0 + even xT
        # chunks, scalar ring carries odd xT chunks + wgu1 — halves the
        # serial ~0.6us/DMA issue latency chain on each ring.
        for i, (a, b) in enumerate(ladder):
            nc.sync.dma_start(wt0[:, a:b], wgu[0, :, a:b])
            for t in range(tc_chunks):
                eng = nc.scalar if t % 2 == 0 else nc.sync
                eng.dma_start(xT_sb[:, t, a:b], xT[t, :, a:b])
            if i == 1 or (i == 0 and len(ladder) == 1):
                nc.scalar.dma_start(sgu_sb[:], sgu[:])
            if i >= 2:
                # trail group 1's weight ladder two blocks behind the fill so
                # its early slabs land before group 1's matmuls need them
                a1, b1 = ladder[i - 2]
                nc.scalar.dma_start(wt1[:, a1:b1], wgu[1, :, a1:b1])
        for a1, b1 in ladder[max(0, len(ladder) - 2):]:
            nc.scalar.dma_start(wt1[:, a1:b1], wgu[1, :, a1:b1])
        # wcomb is epilogue-only (not PE-blocking until group 0's psums must
        # recycle at ~group 2); keep its 0.5MB out of the PE-critical fill
        nc.scalar.dma_start(wc_sb[:], wcomb[:])
        nc.sync.dma_start(sd_sb[:], sd[:])

        h_sb = hpool.tile([P, ki, C], fp16)

        # ---- gate_up matmul + SiLU GLU (combine weight folded in) ----
        for j in range(nj):
            if j == 0:
                wt = wt0
            elif j == 1:
                wt = wt1
            else:
                wt = wpool.tile([P, kh, 256], fp16, tag="wgu")
                nc.sync.dma_start(wt[:], wgu[j])
            # k-major across the batch's (t-chunk, gate/up) psums: each
            # matmul only needs k-slab k of its operands, matching slab
            # arrival order. Batches of <=3 t-chunks keep psum tile demand
            # <=6 of the pool's 8 slots (deadlock-free for any tc_chunks).
            for t0_b in range(0, tc_chunks, 3):
              tb = list(range(t0_b, min(t0_b + 3, tc_chunks)))
              pss = {t: (psum_pool.tile([P, W], fp32, tag="ps", name=f"psg{t}"),
                         psum_pool.tile([P, W], fp32, tag="ps", name=f"psu{t}"))
                     for t in tb}
              for k in range(kh):
                for t in tb:
                    nc.tensor.matmul(pss[t][0][:], wt[:, k, 0:P],
                                     xT_sb[:, t, k],
                                     start=(k == 0), stop=(k == kh - 1))
                    nc.tensor.matmul(pss[t][1][:], wt[:, k, P:2 * P],
                                     xT_sb[:, t, k],
                                     start=(k == 0), stop=(k == kh - 1))
              for t in tb:
                ts = slice(t * W, (t + 1) * W)
                ps_g, ps_u = pss[t]
                # h = sigmoid(g*sg) * g * u * (sg*su) * wcomb
                # (col 2j of sgu holds sg; col 2j+1 holds sg*su)
                act = tmp_pool.tile([P, W], fp32, tag="act")
                nc.scalar.activation(act[:], ps_g[:],
                                     mybir.ActivationFunctionType.Sigmoid,
                                     scale=sgu_sb[:, 2 * j:2 * j + 1])
                m1 = tmp_pool.tile([P, W], fp32, tag="m1")
                nc.vector.tensor_mul(m1[:], act[:], ps_u[:])
                nc.vector.tensor_mul(m1[:], m1[:], ps_g[:])
                nc.vector.tensor_scalar_mul(m1[:], m1[:],
                                            sgu_sb[:, 2 * j + 1:2 * j + 2])
                nc.vector.tensor_tensor(h_sb[:, j, ts], m1[:], wc_sb[:, ts],
                                        mybir.AluOpType.mult)

        # ---- down matmul + per-channel scale ----
        for g in range(ng):
            wdt = wpool.tile([P, ki, 256], fp16, tag="wd")
            nc.sync.dma_start(wdt[:], wd[g])
            for half in range(2):
                m = 2 * g + half
                ot = out_pool.tile([P, C], fp32, tag="ot")
                for t in range(tc_chunks):
                    ts = slice(t * W, (t + 1) * W)
                    ps = psum_pool.tile([P, W], fp32, tag="ps")
                    for k in range(ki):
                        nc.tensor.matmul(ps[:], wdt[:, k, half * P:(half + 1) * P],
                                         h_sb[:, k, ts],
                                         start=(k == 0), stop=(k == ki - 1))
                    nc.vector.tensor_scalar_mul(ot[:, ts], ps[:], sd_sb[:, m:m + 1])
                    nc.scalar.dma_start(out[:, m, ts], ot[:, ts])

    nc.compile()
    return nc


_NC_CACHE = {}


def _get_nc(C, W):
    key = (C, W)
    if key not in _NC_CACHE:
        _NC_CACHE[key] = build_moe_nc(C, W=W)
    return _NC_CACHE[key]


def _prep_core_inputs(e, C, W, hidden, combine, gate_up_w_q, gate_up_scale,
                      down_w_q, down_scale):
    """Build the device input map for expert e. Returns (in_map, token_ids)."""
    ids = np.nonzero(combine[:, e])[0]
    n = len(ids)
    tc_chunks = C // W

    xTf = np.zeros((H, C), np.float16)
    if n:
        xTf[:, :n] = hidden[ids].T.astype(np.float16)
    xT_dev = np.ascontiguousarray(
        xTf.reshape(KH, P, tc_chunks, W).transpose(2, 1, 0, 3))

    wgu_c = (gate_up_w_q[e].astype(np.int16) - 128).astype(np.float16)  # [H, 2I]
    wg = wgu_c[:, :I].reshape(H, NJ, P)
    wu = wgu_c[:, I:].reshape(H, NJ, P)
    pairs = np.concatenate([wg, wu], axis=2)                       # [H, NJ, 256]
    wgu_dev = np.ascontiguousarray(
        pairs.reshape(KH, P, NJ, 256).transpose(2, 1, 0, 3))       # [NJ,128,KH,256]

    wd_c = (down_w_q[e].astype(np.int16) - 128).astype(np.float16)  # [I, H]
    wd_dev = np.ascontiguousarray(
        wd_c.reshape(KI, P, NG, 256).transpose(2, 1, 0, 3))        # [NG,128,KI,256]

    sg = gate_up_scale[e, 0, :I].reshape(NJ, P).astype(np.float32)
    su = gate_up_scale[e, 0, I:].reshape(NJ, P).astype(np.float32)
    sgu_dev = np.empty((P, 2 * NJ), np.float32)
    sgu_dev[:, 0::2] = sg.T
    sgu_dev[:, 1::2] = (sg * su).T

    sd_dev = np.ascontiguousarray(
        down_scale[e, 0].reshape(H // P, P).T.astype(np.float32))  # [128, 32]

    wvec = np.zeros(C, np.float32)
    if n:
        wvec[:n] = combine[ids, e]
    wcomb_dev = np.ascontiguousarray(np.broadcast_to(wvec[None, :], (P, C)))

    return dict(xT=xT_dev, wgu=wgu_dev, wd=wd_dev, sgu=sgu_dev, sd=sd_dev,
                wcomb=wcomb_dev), ids


def host_routing(expert_affinities, expert_index):
    """Top-k affinity normalization -> dense combine matrix [T, E]."""
    T = expert_index.shape[0]
    sel = np.take_along_axis(expert_affinities.astype(np.float32),
                             expert_index, axis=1)
    sel = sel / sel.sum(axis=1, keepdims=True)
    combine = np.zeros((T, E), np.float32)
    np.add.at(combine,
              (np.repeat(np.arange(T), expert_index.shape[1]),
               expert_index.ravel()),
              sel.ravel())
    return combine


def kernel(hidden_states, expert_affinities, gate_up_w_q, gate_up_scale,
           down_w_q, down_scale, expert_index, seq_len=None, **_unused):
    hidden = np.asarray(hidden_states, dtype=np.float32)
    aff = np.asarray(expert_affinities, dtype=np.float32)
    ei = np.asarray(expert_index, dtype=np.int64)
    gq = np.asarray(gate_up_w_q)
    gs = np.asarray(gate_up_scale, dtype=np.float32)
    dq = np.asarray(down_w_q)
    ds = np.asarray(down_scale, dtype=np.float32)
    T = hidden.shape[0]

    combine = host_routing(aff, ei)
    counts = (combine > 0).sum(axis=0)
    cmax = max(2, int(counts.max()))
    tc = max(1, int(math.ceil(cmax / 512)))
    Wc = int(math.ceil(cmax / (2 * tc))) * 2   # even chunk width
    C = tc * Wc

    nc = _get_nc(C, Wc)

    in_maps = []
    all_ids = []
    for e in range(E):
        im, ids = _prep_core_inputs(e, C, Wc, hidden, combine, gq, gs, dq, ds)
        in_maps.append(im)
        all_ids.append(ids)

    res = run_bass_kernel_spmd(nc, in_maps, list(range(E)))

    y = np.zeros((T, H), np.float32)
    for e in range(E):
        ids = all_ids[e]
        if len(ids) == 0:
            continue
        out_dev = res.results[e]["out"]            # [128, 32, C]
        out_full = out_dev.transpose(1, 0, 2).reshape(H, C)
        y[ids] += out_full[:, :len(ids)].T
    return y

